# revision 7
# baseline (speedup 1.0000x reference)
"""Multi-head attention TRN2 kernel (B=2, S=2048, D=1024, H=16).

Sharding (8 cores): B(2) x head-group(2) x query-block(2).
Each core: one batch b, 8 heads, 1024 query rows. The output projection
is a per-head-group partial sum; the host adds the two partials while
gathering (unshard step).

On-chip layout is feature-major: activations are [feature, token] so
every matmul contracts along the partition dim. Host sharding hands each
core pre-transposed contiguous arrays in bf16 (the kernel computes in
bf16 anyway; converting on host removes the on-chip cast phase and
halves HBM traffic).

Softmax: the reference masks scores with -1e-9 (sic), so masked
positions contribute exp(-1e-9) == 1.0f exactly. We multiply the raw
scores by the 0/1 mask in PSUM (masked -> exp(0) == 1, identical
result), so exp(s*m/8) is directly the PV weight; an appended
ones-column on V yields the softmax denominator in the same matmul.
"""

import sys

if "/opt/trn_rl_repo" not in sys.path:
    sys.path.insert(0, "/opt/trn_rl_repo")

import numpy as np
import ml_dtypes

import concourse.bass as bass
import concourse.tile as tile
from concourse import bacc, mybir
from concourse.bass_utils import run_bass_kernel_spmd

F32 = mybir.dt.float32
F32R = mybir.dt.float32r
BF16 = mybir.dt.bfloat16
AF = mybir.ActivationFunctionType
ALU = mybir.AluOpType

B, S, D, H = 2, 2048, 1024, 16
DK = 64
Q = 1024          # query rows per core
DH = 512          # head-group feature dims per core
NPAIR = 4         # head pairs per core
KC = S // 128     # 16 contraction chunks over k tokens
EC = D // 128     # 8 contraction chunks over model dim
QNB = Q // 512    # 2 query n-blocks
SNB = S // 512    # 4 khT n-blocks
HC = DH // 128    # 4 xT partition chunks

_PROGRAM = None


def _build_program():
    nc = bacc.Bacc("TRN2", debug=False, num_devices=8)

    qT = nc.dram_tensor("qT", [D, Q], BF16, kind="ExternalInput")
    kT = nc.dram_tensor("kT", [D, S], BF16, kind="ExternalInput")
    vT = nc.dram_tensor("vT", [D, S], BF16, kind="ExternalInput")
    maskT = nc.dram_tensor("maskT", [S, Q], BF16, kind="ExternalInput")
    wqT = nc.dram_tensor("wqT", [D, DH], BF16, kind="ExternalInput")
    wkT = nc.dram_tensor("wkT", [D, DH], BF16, kind="ExternalInput")
    wvT = nc.dram_tensor("wvT", [D, DH], BF16, kind="ExternalInput")
    woT = nc.dram_tensor("woT", [DH, D], F32R, kind="ExternalInput")
    bqv = nc.dram_tensor("bqv", [128, NPAIR], F32, kind="ExternalInput")
    bkv = nc.dram_tensor("bkv", [128, NPAIR], F32, kind="ExternalInput")
    bvv = nc.dram_tensor("bvv", [1, DH], F32, kind="ExternalInput")
    bov = nc.dram_tensor("bov", [128, EC], F32, kind="ExternalInput")
    out = nc.dram_tensor("out", [D, Q], F32, kind="ExternalOutput")

    with tile.TileContext(nc) as tc:
        _emit(nc, tc, qT, kT, vT, maskT, wqT, wkT, wvT, woT, bqv, bkv, bvv, bov, out)
    nc.compile()
    return nc


def _emit(nc, tc, qT, kT, vT, maskT, wqT, wkT, wvT, woT, bqv, bkv, bvv, bov, out):
    from contextlib import ExitStack

    ctx = ExitStack()
    with ctx:
        consts = ctx.enter_context(tc.tile_pool(name="consts", bufs=1))
        big = ctx.enter_context(tc.tile_pool(name="big", bufs=1))
        work = ctx.enter_context(tc.tile_pool(name="work", bufs=2))
        pp = ctx.enter_context(tc.tile_pool(name="pp", bufs=2, space="PSUM"))
        pvp = ctx.enter_context(tc.tile_pool(name="pvp", bufs=1, space="PSUM"))
        kqd_cm = tc.tile_pool(name="kqd", bufs=1, side="right")
        kqd = kqd_cm.__enter__()

        # ---- small constants ----
        t_bq = consts.tile([128, NPAIR], F32)
        t_bk = consts.tile([128, NPAIR], F32)
        t_bo = consts.tile([128, EC], F32)
        nc.sync.dma_start(out=t_bq, in_=bqv[:, :])
        nc.sync.dma_start(out=t_bk, in_=bkv[:, :])
        nc.sync.dma_start(out=t_bo, in_=bov[:, :])
        vbias = consts.tile([128, DH], F32)
        bv_b = bass.AP(tensor=bvv.ap().tensor, offset=0, ap=[[0, 128]] + bvv.ap().ap[1:])
        nc.gpsimd.dma_start(out=vbias, in_=bv_b)
        ones_f0 = consts.tile([1, 64], F32)
        nc.vector.memset(ones_f0, 1.0)
        ones_r = consts.tile([1, 64], F32R)
        nc.vector.tensor_copy(out=ones_r, in_=ones_f0)

        # ---- resident activation tensors ----
        khT = [big.tile([128, S], BF16, name=f"khT{i}") for i in range(NPAIR)]
        qhT = [big.tile([128, Q], BF16, name=f"qhT{i}") for i in range(NPAIR)]
        vh_aug = [big.tile([128, 8, 65], BF16, name=f"vha{i}") for i in range(KC)]
        mb = [big.tile([128, Q], BF16, name=f"mb{i}") for i in range(KC)]

        # ---- k/q weights + inputs, DMA'd directly as bf16 ----
        kTb = [kqd.tile([128, S], BF16, name=f"kTb{i}") for i in range(EC)]
        qTb = [kqd.tile([128, Q], BF16, name=f"qTb{i}") for i in range(EC)]
        wkb = [kqd.tile([128, DH], BF16, name=f"wkb{i}") for i in range(EC)]
        wqb = [kqd.tile([128, DH], BF16, name=f"wqb{i}") for i in range(EC)]

        for ec in range(EC):
            nc.sync.dma_start(out=wkb[ec], in_=wkT[ec * 128:(ec + 1) * 128, :])
        # kT loaded in column halves, half-major so attention can start early
        for sh in range(2):
            for ec in range(EC):
                nc.sync.dma_start(
                    out=kTb[ec][:, sh * Q:(sh + 1) * Q],
                    in_=kT[ec * 128:(ec + 1) * 128, sh * Q:(sh + 1) * Q])
        for ec in range(EC):
            nc.sync.dma_start(out=wqb[ec], in_=wqT[ec * 128:(ec + 1) * 128, :])
        for ec in range(EC):
            nc.sync.dma_start(out=qTb[ec], in_=qT[ec * 128:(ec + 1) * 128, :])

        def proj_pair(p):
            for sb in range(SNB):
                ps = pp.tile([128, 512], F32, tag="pp", name=f"khps{p}_{sb}")
                for ec in range(EC):
                    nc.tensor.matmul(
                        ps[:, :], wkb[ec][:, p * 128:(p + 1) * 128],
                        kTb[ec][:, sb * 512:(sb + 1) * 512],
                        start=(ec == 0), stop=(ec == EC - 1))
                nc.scalar.add(
                    out=khT[p][:, sb * 512:(sb + 1) * 512], in_=ps[:, :],
                    add=t_bk[:, p:p + 1])
            for qb in range(QNB):
                ps = pp.tile([128, 512], F32, tag="pp", name=f"qhps{p}_{qb}")
                for ec in range(EC):
                    nc.tensor.matmul(
                        ps[:, :], wqb[ec][:, p * 128:(p + 1) * 128],
                        qTb[ec][:, qb * 512:(qb + 1) * 512],
                        start=(ec == 0), stop=(ec == EC - 1))
                nc.scalar.add(
                    out=qhT[p][:, qb * 512:(qb + 1) * 512], in_=ps[:, :],
                    add=t_bq[:, p:p + 1])

        proj_pair(0)

        # ---- masks: direct bf16 DMA on the gpsimd queue ----
        for kc in range(KC):
            nc.gpsimd.dma_start(out=mb[kc], in_=maskT[kc * 128:(kc + 1) * 128, :])

        # ---- vh projection ----
        with (
            tc.tile_pool(name="vs", bufs=2) as vs,
            tc.tile_pool(name="vtb", bufs=1) as vtbp,
            tc.tile_pool(name="pvh", bufs=3, space="PSUM") as pvh,
        ):
            wvb = [vtbp.tile([128, DH], BF16, name=f"wvb{i}") for i in range(EC)]
            for ec in range(EC):
                nc.gpsimd.dma_start(out=wvb[ec], in_=wvT[ec * 128:(ec + 1) * 128, :])
            for sc in range(KC):
                nc.vector.memset(vh_aug[sc][:, :, 64:65], 1.0)
            for qtr in range(4):
                vq = [vs.tile([128, 512], BF16, tag=f"vq{i}", name=f"vq{qtr}_{i}")
                      for i in range(EC)]
                for ec in range(EC):
                    nc.gpsimd.dma_start(
                        out=vq[ec],
                        in_=vT[ec * 128:(ec + 1) * 128, qtr * 512:(qtr + 1) * 512])
                for si in range(4):
                    sc = qtr * 4 + si
                    ps = pvh.tile([128, 512], F32, tag="vps", name=f"vps{sc}")
                    for ec in range(EC):
                        nc.tensor.matmul(
                            ps[:, :], vq[ec][:, si * 128:(si + 1) * 128], wvb[ec][:, :],
                            start=(ec == 0), stop=(ec == EC - 1))
                    nc.vector.tensor_tensor(
                        out=vh_aug[sc][:, :, 0:64],
                        in0=ps.rearrange("p (h d) -> p h d", h=8),
                        in1=vbias.rearrange("p (h d) -> p h d", h=8),
                        op=ALU.add)

        # ---- attention ----
        xT = [big.tile([128, Q], F32R, name=f"xT{i}") for i in range(HC)]
        with tc.tile_pool(name="scp", bufs=2, space="PSUM") as scp:
            for p in range(NPAIR):
                for nb in range(QNB):
                    pv0 = pvp.tile([65, 512], F32, tag="pv0", name=f"pv0_{p}{nb}")
                    pv1 = pvp.tile([65, 512], F32, tag="pv1", name=f"pv1_{p}{nb}")
                    for kc in range(KC):
                        sc_ps = scp.tile([128, 2, 512], F32, tag="sc", name=f"sc{p}_{nb}_{kc}")
                        nc.tensor.matmul(
                            sc_ps[:, 0, :],
                            khT[p][0:64, kc * 128:(kc + 1) * 128],
                            qhT[p][0:64, nb * 512:(nb + 1) * 512],
                            start=True, stop=True)
                        nc.tensor.matmul(
                            sc_ps[:, 1, :],
                            khT[p][64:128, kc * 128:(kc + 1) * 128],
                            qhT[p][64:128, nb * 512:(nb + 1) * 512],
                            start=True, stop=True, tile_position=(64, 0))
                        # mask in place: masked scores -> 0 -> exp(0) == 1,
                        # matching the reference's exp(-1e-9) == 1.0f
                        m_in = mb[kc][:, None, nb * 512:(nb + 1) * 512].to_broadcast(
                            [128, 2, 512])
                        nc.vector.tensor_tensor(
                            out=sc_ps[:, :, :], in0=sc_ps[:, :, :], in1=m_in,
                            op=ALU.mult)
                        E = work.tile([128, 2, 512], BF16, tag="E", bufs=4,
                                      name=f"E{p}_{nb}_{kc}")
                        nc.scalar.activation(
                            out=E.rearrange("p h q -> p (h q)"),
                            in_=sc_ps.rearrange("p h q -> p (h q)"),
                            func=AF.Exp, scale=0.125)
                        nc.tensor.matmul(
                            pv0[:, :], vh_aug[kc][:, 2 * p, :], E[:, 0, :],
                            start=(kc == 0), stop=(kc == KC - 1))
                        nc.tensor.matmul(
                            pv1[:, :], vh_aug[kc][:, 2 * p + 1, :], E[:, 1, :],
                            start=(kc == 0), stop=(kc == KC - 1))
                    for hh, pv in ((0, pv0), (1, pv1)):
                        h = 2 * p + hh
                        den = work.tile([1, 512], F32, tag="den", bufs=1,
                                        name=f"den{p}{nb}{hh}")
                        nc.scalar.copy(out=den, in_=pv[64:65, :])
                        rcp_f = work.tile([1, 512], F32, tag="rcpf", bufs=1,
                                          name=f"rcpf{p}{nb}{hh}")
                        nc.vector.reciprocal_approx_fast(out=rcp_f, in_=den)
                        rcp = work.tile([1, 512], F32R, tag="rcp", bufs=1,
                                        name=f"rcp{p}{nb}{hh}")
                        with nc.allow_low_precision(reason="f32r denominator broadcast"):
                            nc.vector.tensor_copy(out=rcp, in_=rcp_f)
                        br_ps = pp.tile([64, 512], F32, tag="pp", name=f"br{p}{nb}{hh}")
                        nc.tensor.matmul(br_ps[:, :], ones_r, rcp, start=True, stop=True)
                        br_sb = work.tile([64, 512], F32, tag="brs", bufs=2,
                                          name=f"brs{p}{nb}{hh}")
                        nc.scalar.copy(out=br_sb, in_=br_ps[:, :])
                        nc.vector.tensor_tensor(
                            out=xT[h // 2][(h % 2) * 64:(h % 2) * 64 + 64,
                                           nb * 512:(nb + 1) * 512],
                            in0=pv[0:64, :], in1=br_sb, op=ALU.mult)
                if p + 1 < NPAIR:
                    proj_pair(p + 1)

            kqd_cm.__exit__(None, None, None)

            # ---- output projection ----
            with tc.tile_pool(name="wop", bufs=1) as wop:
                wob = [wop.tile([128, Q], F32R, name=f"wob{i}") for i in range(HC)]
                for hc in range(HC):
                    nc.sync.dma_start(out=wob[hc], in_=woT[hc * 128:(hc + 1) * 128, :])
                for dc in range(EC):
                    for nb in range(QNB):
                        ps = pp.tile([128, 512], F32, tag="pp", name=f"ops{dc}_{nb}")
                        for hc in range(HC):
                            nc.tensor.matmul(
                                ps[:, :], wob[hc][:, dc * 128:(dc + 1) * 128],
                                xT[hc][:, nb * 512:(nb + 1) * 512],
                                start=(hc == 0), stop=(hc == HC - 1))
                        o_sb = work.tile([128, 512], F32, tag="osb", bufs=2,
                                         name=f"osb{dc}_{nb}")
                        nc.scalar.add(out=o_sb, in_=ps[:, :], add=t_bo[:, dc:dc + 1])
                        nc.sync.dma_start(
                            out=out[dc * 128:(dc + 1) * 128, nb * 512:(nb + 1) * 512],
                            in_=o_sb)


def _get_program():
    global _PROGRAM
    if _PROGRAM is None:
        _PROGRAM = _build_program()
    return _PROGRAM


def kernel(q, k, v, mask, Wq, bq, Wk, bk, Wv, bv, Wo, bo, _trace=False):
    bf16 = ml_dtypes.bfloat16
    q = np.asarray(q, np.float32)
    k = np.asarray(k, np.float32)
    v = np.asarray(v, np.float32)
    Wq = np.asarray(Wq, np.float32)
    Wk = np.asarray(Wk, np.float32)
    Wv = np.asarray(Wv, np.float32)
    Wo = np.asarray(Wo, np.float32)
    bq = np.asarray(bq, np.float32)
    bk = np.asarray(bk, np.float32)
    bv = np.asarray(bv, np.float32)
    bo = np.asarray(bo, np.float32)
    mask_f = np.asarray(mask).astype(np.float32)

    nc = _get_program()

    # per-batch transposed bf16 copies shared by the 4 cores of each batch
    kT_b = [np.ascontiguousarray(k[b].T).astype(bf16) for b in range(B)]
    vT_b = [np.ascontiguousarray(v[b].T).astype(bf16) for b in range(B)]
    wqT_f = np.ascontiguousarray(Wq.T).astype(bf16)
    wkT_f = np.ascontiguousarray(Wk.T).astype(bf16)
    wvT_f = np.ascontiguousarray(Wv.T).astype(bf16)
    woT_f = np.ascontiguousarray(Wo.T)

    in_maps = []
    for c in range(8):
        b, hg, sq = c // 4, (c // 2) % 2, c % 2
        hsl = slice(hg * DH, (hg + 1) * DH)
        in_maps.append({
            "qT": np.ascontiguousarray(q[b, sq * Q:(sq + 1) * Q, :].T).astype(bf16),
            "kT": kT_b[b],
            "vT": vT_b[b],
            "maskT": np.ascontiguousarray(
                mask_f[b, 0, sq * Q:(sq + 1) * Q, :].T).astype(bf16),
            "wqT": np.ascontiguousarray(wqT_f[:, hsl]),
            "wkT": np.ascontiguousarray(wkT_f[:, hsl]),
            "wvT": np.ascontiguousarray(wvT_f[:, hsl]),
            "woT": np.ascontiguousarray(woT_f[hsl, :]),
            "bqv": np.ascontiguousarray(bq[hsl].reshape(NPAIR, 128).T),
            "bkv": np.ascontiguousarray(bk[hsl].reshape(NPAIR, 128).T),
            "bvv": np.ascontiguousarray(bv[hsl].reshape(1, DH)),
            "bov": np.ascontiguousarray(
                (bo if hg == 0 else np.zeros_like(bo)).reshape(EC, 128).T),
        })

    kw = {}
    if _trace:
        kw = dict(trace=True, trace_cores=list(range(8)))
    res = run_bass_kernel_spmd(nc, in_maps, core_ids=list(range(8)), **kw)
    kernel._last_res = res

    outp = np.empty((B, S, D), np.float32)
    for b in range(B):
        for sq in range(2):
            c0 = b * 4 + sq
            c1 = b * 4 + 2 + sq
            outp[b, sq * Q:(sq + 1) * Q, :] = (
                res.results[c0]["out"] + res.results[c1]["out"]).T
    if _trace:
        return outp, res
    return outp


# revision 9
# speedup vs baseline: 1.1132x; 1.1132x over previous
"""Multi-head attention TRN2 kernel (B=2, S=2048, D=1024, H=16).

Sharding (8 cores): B(2) x head-group(2) x query-block(2).
Each core: one batch b, 8 heads, 1024 query rows. The output projection
is a per-head-group partial sum; the host adds the two partials while
gathering (unshard step).

On-chip layout is feature-major: activations are [feature, token] so
every matmul contracts along the partition dim. Host sharding hands each
core pre-transposed contiguous arrays in bf16 (the kernel computes in
bf16 anyway; converting on host removes the on-chip cast phase and
halves HBM traffic).

Softmax: the reference masks scores with -1e-9 (sic), so masked
positions contribute exp(-1e-9) == 1.0f exactly. We multiply the raw
scores by the 0/1 mask in PSUM (masked -> exp(0) == 1, identical
result), so exp(s*m/8) is directly the PV weight; an appended
ones-column on V yields the softmax denominator in the same matmul.
"""

import sys

if "/opt/trn_rl_repo" not in sys.path:
    sys.path.insert(0, "/opt/trn_rl_repo")

import numpy as np
import ml_dtypes

import concourse.bass as bass
import concourse.tile as tile
from concourse import bacc, mybir
from concourse.bass_utils import run_bass_kernel_spmd

F32 = mybir.dt.float32
F32R = mybir.dt.float32r
BF16 = mybir.dt.bfloat16
AF = mybir.ActivationFunctionType
ALU = mybir.AluOpType

B, S, D, H = 2, 2048, 1024, 16
DK = 64
Q = 1024          # query rows per core
DH = 512          # head-group feature dims per core
NPAIR = 4         # head pairs per core
KC = S // 128     # 16 contraction chunks over k tokens
EC = D // 128     # 8 contraction chunks over model dim
QNB = Q // 512    # 2 query n-blocks
SNB = S // 512    # 4 khT n-blocks
HC = DH // 128    # 4 xT partition chunks

_PROGRAM = None


def _build_program():
    nc = bacc.Bacc("TRN2", debug=False, num_devices=8)

    qT = nc.dram_tensor("qT", [D, Q], BF16, kind="ExternalInput")
    kT = nc.dram_tensor("kT", [D, S], BF16, kind="ExternalInput")
    vT = nc.dram_tensor("vT", [D, S], BF16, kind="ExternalInput")
    maskT = nc.dram_tensor("maskT", [S, Q], BF16, kind="ExternalInput")
    wqT = nc.dram_tensor("wqT", [D, DH], BF16, kind="ExternalInput")
    wkT = nc.dram_tensor("wkT", [D, DH], BF16, kind="ExternalInput")
    wvT = nc.dram_tensor("wvT", [D, DH], BF16, kind="ExternalInput")
    woT = nc.dram_tensor("woT", [DH, D], F32R, kind="ExternalInput")
    bqv = nc.dram_tensor("bqv", [128, NPAIR], F32, kind="ExternalInput")
    bkv = nc.dram_tensor("bkv", [128, NPAIR], F32, kind="ExternalInput")
    bvv = nc.dram_tensor("bvv", [1, DH], F32, kind="ExternalInput")
    bov = nc.dram_tensor("bov", [128, EC], F32, kind="ExternalInput")
    out = nc.dram_tensor("out", [D, Q], F32, kind="ExternalOutput")

    with tile.TileContext(nc) as tc:
        _emit(nc, tc, qT, kT, vT, maskT, wqT, wkT, wvT, woT, bqv, bkv, bvv, bov, out)
    nc.compile()
    return nc


def _emit(nc, tc, qT, kT, vT, maskT, wqT, wkT, wvT, woT, bqv, bkv, bvv, bov, out):
    from contextlib import ExitStack

    ctx = ExitStack()
    with ctx:
        consts = ctx.enter_context(tc.tile_pool(name="consts", bufs=1))
        big = ctx.enter_context(tc.tile_pool(name="big", bufs=1))
        work = ctx.enter_context(tc.tile_pool(name="work", bufs=2))
        pp = ctx.enter_context(tc.tile_pool(name="pp", bufs=2, space="PSUM"))
        pvp = ctx.enter_context(tc.tile_pool(name="pvp", bufs=1, space="PSUM"))
        kqd_cm = tc.tile_pool(name="kqd", bufs=1, side="right")
        kqd = kqd_cm.__enter__()

        # ---- small constants ----
        t_bq = consts.tile([128, NPAIR], F32)
        t_bk = consts.tile([128, NPAIR], F32)
        t_bo = consts.tile([128, EC], F32)
        nc.sync.dma_start(out=t_bq, in_=bqv[:, :])
        nc.sync.dma_start(out=t_bk, in_=bkv[:, :])
        nc.sync.dma_start(out=t_bo, in_=bov[:, :])
        vbias = consts.tile([128, DH], F32)
        bv_b = bass.AP(tensor=bvv.ap().tensor, offset=0, ap=[[0, 128]] + bvv.ap().ap[1:])
        nc.gpsimd.dma_start(out=vbias, in_=bv_b)
        ones_f0 = consts.tile([1, 64], F32)
        nc.vector.memset(ones_f0, 1.0)
        ones_r = consts.tile([1, 64], F32R)
        nc.vector.tensor_copy(out=ones_r, in_=ones_f0)

        # ---- resident activation tensors ----
        khT = [big.tile([128, S], BF16, name=f"khT{i}") for i in range(NPAIR)]
        qhT = [big.tile([128, Q], BF16, name=f"qhT{i}") for i in range(NPAIR)]
        vh_aug = [big.tile([128, 8, 65], BF16, name=f"vha{i}") for i in range(KC)]
        mb = [big.tile([128, Q], BF16, name=f"mb{i}") for i in range(KC)]

        # ---- k/q weights + inputs, DMA'd directly as bf16 ----
        kTb = [kqd.tile([128, S], BF16, name=f"kTb{i}") for i in range(EC)]
        qTb = [kqd.tile([128, Q], BF16, name=f"qTb{i}") for i in range(EC)]
        wkb = [kqd.tile([128, DH], BF16, name=f"wkb{i}") for i in range(EC)]
        wqb = [kqd.tile([128, DH], BF16, name=f"wqb{i}") for i in range(EC)]

        for ec in range(EC):
            nc.sync.dma_start(out=wkb[ec], in_=wkT[ec * 128:(ec + 1) * 128, :])
        # kT loaded in column halves, half-major so attention can start early
        for sh in range(2):
            for ec in range(EC):
                nc.sync.dma_start(
                    out=kTb[ec][:, sh * Q:(sh + 1) * Q],
                    in_=kT[ec * 128:(ec + 1) * 128, sh * Q:(sh + 1) * Q])
        for ec in range(EC):
            nc.sync.dma_start(out=wqb[ec], in_=wqT[ec * 128:(ec + 1) * 128, :])
        for ec in range(EC):
            nc.sync.dma_start(out=qTb[ec], in_=qT[ec * 128:(ec + 1) * 128, :])

        def proj_pair(p):
            for sb in range(SNB):
                ps = pp.tile([128, 512], F32, tag="pp", name=f"khps{p}_{sb}")
                for ec in range(EC):
                    nc.tensor.matmul(
                        ps[:, :], wkb[ec][:, p * 128:(p + 1) * 128],
                        kTb[ec][:, sb * 512:(sb + 1) * 512],
                        start=(ec == 0), stop=(ec == EC - 1))
                nc.scalar.add(
                    out=khT[p][:, sb * 512:(sb + 1) * 512], in_=ps[:, :],
                    add=t_bk[:, p:p + 1])
            for qb in range(QNB):
                ps = pp.tile([128, 512], F32, tag="pp", name=f"qhps{p}_{qb}")
                for ec in range(EC):
                    nc.tensor.matmul(
                        ps[:, :], wqb[ec][:, p * 128:(p + 1) * 128],
                        qTb[ec][:, qb * 512:(qb + 1) * 512],
                        start=(ec == 0), stop=(ec == EC - 1))
                nc.scalar.add(
                    out=qhT[p][:, qb * 512:(qb + 1) * 512], in_=ps[:, :],
                    add=t_bq[:, p:p + 1])

        proj_pair(0)

        # ---- masks: direct bf16 DMA on the gpsimd queue ----
        for kc in range(KC):
            nc.gpsimd.dma_start(out=mb[kc], in_=maskT[kc * 128:(kc + 1) * 128, :])

        # ---- vh projection ----
        with (
            tc.tile_pool(name="vs", bufs=2) as vs,
            tc.tile_pool(name="vtb", bufs=1) as vtbp,
            tc.tile_pool(name="pvh", bufs=3, space="PSUM") as pvh,
        ):
            wvb = [vtbp.tile([128, DH], BF16, name=f"wvb{i}") for i in range(EC)]
            for ec in range(EC):
                nc.gpsimd.dma_start(out=wvb[ec], in_=wvT[ec * 128:(ec + 1) * 128, :])
            for sc in range(KC):
                nc.vector.memset(vh_aug[sc][:, :, 64:65], 1.0)
            for qtr in range(4):
                vq = [vs.tile([128, 512], BF16, tag=f"vq{i}", name=f"vq{qtr}_{i}")
                      for i in range(EC)]
                for ec in range(EC):
                    nc.gpsimd.dma_start(
                        out=vq[ec],
                        in_=vT[ec * 128:(ec + 1) * 128, qtr * 512:(qtr + 1) * 512])
                for si in range(4):
                    sc = qtr * 4 + si
                    ps = pvh.tile([128, 512], F32, tag="vps", name=f"vps{sc}")
                    for ec in range(EC):
                        nc.tensor.matmul(
                            ps[:, :], vq[ec][:, si * 128:(si + 1) * 128], wvb[ec][:, :],
                            start=(ec == 0), stop=(ec == EC - 1))
                    nc.vector.tensor_tensor(
                        out=vh_aug[sc][:, :, 0:64],
                        in0=ps.rearrange("p (h d) -> p h d", h=8),
                        in1=vbias.rearrange("p (h d) -> p h d", h=8),
                        op=ALU.add)

        # ---- attention ----
        xT = [big.tile([128, Q], F32R, name=f"xT{i}") for i in range(HC)]
        with tc.tile_pool(name="scp", bufs=4, space="PSUM") as scp:
            for p in range(NPAIR):
                for nb in range(QNB):
                    pv0 = pvp.tile([65, 512], F32, tag="pv0", name=f"pv0_{p}{nb}")
                    pv1 = pvp.tile([65, 512], F32, tag="pv1", name=f"pv1_{p}{nb}")
                    for kc in range(KC):
                        sc0 = scp.tile([128, 512], F32, tag="sc", name=f"sc0_{p}_{nb}_{kc}")
                        sc1 = scp.tile([128, 512], F32, tag="sc", name=f"sc1_{p}_{nb}_{kc}")
                        nc.tensor.matmul(
                            sc0[:, :],
                            khT[p][0:64, kc * 128:(kc + 1) * 128],
                            qhT[p][0:64, nb * 512:(nb + 1) * 512],
                            start=True, stop=True)
                        nc.tensor.matmul(
                            sc1[:, :],
                            khT[p][64:128, kc * 128:(kc + 1) * 128],
                            qhT[p][64:128, nb * 512:(nb + 1) * 512],
                            start=True, stop=True, tile_position=(64, 0))
                        # mask in place: masked scores -> 0 -> exp(0) == 1,
                        # matching the reference's exp(-1e-9) == 1.0f
                        m_in = mb[kc][:, nb * 512:(nb + 1) * 512]
                        for hh, sc_ps in ((0, sc0), (1, sc1)):
                            nc.vector.tensor_tensor(
                                out=sc_ps[:, :], in0=sc_ps[:, :], in1=m_in,
                                op=ALU.mult)
                            E = work.tile([128, 512], BF16, tag="E", bufs=6,
                                          name=f"E{p}_{nb}_{kc}_{hh}")
                            nc.scalar.activation(
                                out=E, in_=sc_ps[:, :], func=AF.Exp, scale=0.125)
                            nc.tensor.matmul(
                                (pv0, pv1)[hh][:, :], vh_aug[kc][:, 2 * p + hh, :],
                                E[:, :],
                                start=(kc == 0), stop=(kc == KC - 1))
                    for hh, pv in ((0, pv0), (1, pv1)):
                        h = 2 * p + hh
                        den = work.tile([1, 512], F32, tag="den", bufs=1,
                                        name=f"den{p}{nb}{hh}")
                        nc.scalar.copy(out=den, in_=pv[64:65, :])
                        rcp_f = work.tile([1, 512], F32, tag="rcpf", bufs=1,
                                          name=f"rcpf{p}{nb}{hh}")
                        nc.vector.reciprocal_approx_fast(out=rcp_f, in_=den)
                        br_sb = work.tile([64, 512], F32, tag="brs", bufs=2,
                                          name=f"brs{p}{nb}{hh}")
                        nc.gpsimd.partition_broadcast(br_sb, rcp_f)
                        nc.vector.tensor_tensor(
                            out=xT[h // 2][(h % 2) * 64:(h % 2) * 64 + 64,
                                           nb * 512:(nb + 1) * 512],
                            in0=pv[0:64, :], in1=br_sb, op=ALU.mult)
                if p + 1 < NPAIR:
                    proj_pair(p + 1)

            kqd_cm.__exit__(None, None, None)

            # ---- output projection ----
            with tc.tile_pool(name="wop", bufs=1) as wop:
                wob = [wop.tile([128, Q], F32R, name=f"wob{i}") for i in range(HC)]
                for hc in range(HC):
                    nc.sync.dma_start(out=wob[hc], in_=woT[hc * 128:(hc + 1) * 128, :])
                for dc in range(EC):
                    for nb in range(QNB):
                        ps = pp.tile([128, 512], F32, tag="pp", name=f"ops{dc}_{nb}")
                        for hc in range(HC):
                            nc.tensor.matmul(
                                ps[:, :], wob[hc][:, dc * 128:(dc + 1) * 128],
                                xT[hc][:, nb * 512:(nb + 1) * 512],
                                start=(hc == 0), stop=(hc == HC - 1))
                        o_sb = work.tile([128, 512], F32, tag="osb", bufs=2,
                                         name=f"osb{dc}_{nb}")
                        nc.scalar.add(out=o_sb, in_=ps[:, :], add=t_bo[:, dc:dc + 1])
                        nc.sync.dma_start(
                            out=out[dc * 128:(dc + 1) * 128, nb * 512:(nb + 1) * 512],
                            in_=o_sb)


def _get_program():
    global _PROGRAM
    if _PROGRAM is None:
        _PROGRAM = _build_program()
    return _PROGRAM


def kernel(q, k, v, mask, Wq, bq, Wk, bk, Wv, bv, Wo, bo, _trace=False):
    bf16 = ml_dtypes.bfloat16
    q = np.asarray(q, np.float32)
    k = np.asarray(k, np.float32)
    v = np.asarray(v, np.float32)
    Wq = np.asarray(Wq, np.float32)
    Wk = np.asarray(Wk, np.float32)
    Wv = np.asarray(Wv, np.float32)
    Wo = np.asarray(Wo, np.float32)
    bq = np.asarray(bq, np.float32)
    bk = np.asarray(bk, np.float32)
    bv = np.asarray(bv, np.float32)
    bo = np.asarray(bo, np.float32)
    mask_f = np.asarray(mask).astype(np.float32)

    nc = _get_program()

    # per-batch transposed bf16 copies shared by the 4 cores of each batch
    kT_b = [np.ascontiguousarray(k[b].T).astype(bf16) for b in range(B)]
    vT_b = [np.ascontiguousarray(v[b].T).astype(bf16) for b in range(B)]
    wqT_f = np.ascontiguousarray(Wq.T).astype(bf16)
    wkT_f = np.ascontiguousarray(Wk.T).astype(bf16)
    wvT_f = np.ascontiguousarray(Wv.T).astype(bf16)
    woT_f = np.ascontiguousarray(Wo.T)

    in_maps = []
    for c in range(8):
        b, hg, sq = c // 4, (c // 2) % 2, c % 2
        hsl = slice(hg * DH, (hg + 1) * DH)
        in_maps.append({
            "qT": np.ascontiguousarray(q[b, sq * Q:(sq + 1) * Q, :].T).astype(bf16),
            "kT": kT_b[b],
            "vT": vT_b[b],
            "maskT": np.ascontiguousarray(
                mask_f[b, 0, sq * Q:(sq + 1) * Q, :].T).astype(bf16),
            "wqT": np.ascontiguousarray(wqT_f[:, hsl]),
            "wkT": np.ascontiguousarray(wkT_f[:, hsl]),
            "wvT": np.ascontiguousarray(wvT_f[:, hsl]),
            "woT": np.ascontiguousarray(woT_f[hsl, :]),
            "bqv": np.ascontiguousarray(bq[hsl].reshape(NPAIR, 128).T),
            "bkv": np.ascontiguousarray(bk[hsl].reshape(NPAIR, 128).T),
            "bvv": np.ascontiguousarray(bv[hsl].reshape(1, DH)),
            "bov": np.ascontiguousarray(
                (bo if hg == 0 else np.zeros_like(bo)).reshape(EC, 128).T),
        })

    kw = {}
    if _trace:
        kw = dict(trace=True, trace_cores=list(range(8)))
    res = run_bass_kernel_spmd(nc, in_maps, core_ids=list(range(8)), **kw)
    kernel._last_res = res

    outp = np.empty((B, S, D), np.float32)
    for b in range(B):
        for sq in range(2):
            c0 = b * 4 + sq
            c1 = b * 4 + 2 + sq
            outp[b, sq * Q:(sq + 1) * Q, :] = (
                res.results[c0]["out"] + res.results[c1]["out"]).T
    if _trace:
        return outp, res
    return outp
